# revision 52
# baseline (speedup 1.0000x reference)
"""Bass/Tile TRN2 kernel for a non-local attention block (BaseNonLocalBlock).

Contract: kernel(**inputs) takes the FULL inputs of the nn.Module problem
(B=1, D=256, H=4, N=4096) and returns the FULL output [1, 256, 4096].

Sharding: query columns of the N x N attention are split across the 8
NeuronCores (512 queries per core). K/V projections are computed
redundantly on every core (cheap); each core produces its own output
column slice and the host concatenates.

Per-core algorithm (flash-attention style, scores never hit HBM):
  Q = (Wq/8) @ xq                      [256, 512]  (1/sqrt(DH) folded in)
  K = Wk @ x                           [256, 4096]
  V_T = x^T @ Wv^T (+ones col/head)    [4096, 4*68] (denominator trick)
  loop over key chunks it (32 x 128 keys), head pair hp:
    S_T = K_h[:, it]^T @ Q_h           [128, 2, 512] psum (two row-packed
                                        matmuls run concurrently in the
                                        upper/lower PE sub-arrays)
    el  = S_T * spt_rep                one flat [128,1024] DVE mult (spt is
                                        DMA'd twice so no broadcast AP)
    E   = exp(el)                      ACT exp, or GPSIMD exp2 bit-trick
                                        (int16(el*184.66+16249) bitcast bf16)
                                        on a subset of tiles to offload ACT
    msg_h += V_T[it,h]^T @ E           [65, 512] psum accum; row 64 = denom
  Projections are interleaved with the attention loop (proj block ib on the
  PE while the elementwise engines process block ib-1) so the DVE -- the
  critical engine -- starts within ~4us.
  norm: recip(denominator row) on [1,512], gpsimd partition_broadcast,
  one DVE mult per head.
  out = xq + W3 @ relu(bn2(W2 @ relu(bn1(W1 @ msg))))  (BN folded into W;
  residual added on the PE via an identity matmul into the last psum).
"""

import numpy as np
from contextlib import ExitStack

D = 256
N = 4096
NQ = 512          # queries per core
H = 4
DH = 64
NCORES = 8
NIT = N // 128    # 32 key chunks
VTS = 68          # padded per-head stride in the V_T-aug tile

A16 = 184.6617    # 128*log2(e) for the exp2 bit-trick
B16 = 16256.0 - 7.0

# tiles (it, hp) with (2*it + hp) % GPS_MOD == GPS_PHASE use the GPSIMD
# bit-trick exp instead of ACT exp (load balancing)
GPS_MOD = 2
GPS_PHASE = 1

_CACHE = {}


def _build(has_bq, has_bk, has_bv, has_b3):
    import concourse.bass as bass
    import concourse.tile as tile
    from concourse import bacc, mybir

    F32 = mybir.dt.float32
    BF16 = mybir.dt.bfloat16
    I16 = mybir.dt.int16
    Id = mybir.ActivationFunctionType.Identity
    Exp = mybir.ActivationFunctionType.Exp
    Relu = mybir.ActivationFunctionType.Relu
    Mu = mybir.AluOpType.mult
    Ad = mybir.AluOpType.add

    nc = bacc.Bacc("TRN2", target_bir_lowering=False, debug=False,
                   num_devices=NCORES)

    x_d = nc.dram_tensor("x", [D, N], BF16, kind="ExternalInput").ap()
    xq_d = nc.dram_tensor("xq", [D, NQ], BF16, kind="ExternalInput").ap()
    spt_d = nc.dram_tensor("sptd", [N, 2 * NQ], BF16, kind="ExternalInput").ap()
    wqt_d = nc.dram_tensor("wqt", [D, D], BF16, kind="ExternalInput").ap()
    wkt_d = nc.dram_tensor("wkt", [D, D], BF16, kind="ExternalInput").ap()
    wvt_d = nc.dram_tensor("wvt", [D, D], BF16, kind="ExternalInput").ap()
    w1t_d = nc.dram_tensor("w1t", [D, 128], BF16, kind="ExternalInput").ap()
    w2t_d = nc.dram_tensor("w2t", [128, 128], BF16, kind="ExternalInput").ap()
    w3t_d = nc.dram_tensor("w3t", [128, D], BF16, kind="ExternalInput").ap()
    idn_d = nc.dram_tensor("idn", [128, 128], BF16, kind="ExternalInput").ap()
    bq_d = nc.dram_tensor("bq2", [128, 2], F32, kind="ExternalInput").ap()
    bk_d = nc.dram_tensor("bk2", [128, 2], F32, kind="ExternalInput").ap()
    bv_d = nc.dram_tensor("bv2", [128, 2], F32, kind="ExternalInput").ap()
    b1_d = nc.dram_tensor("b1f", [128, 1], F32, kind="ExternalInput").ap()
    b2_d = nc.dram_tensor("b2f", [128, 1], F32, kind="ExternalInput").ap()
    b3_d = nc.dram_tensor("b32", [128, 2], F32, kind="ExternalInput").ap()
    out_d = nc.dram_tensor("out", [D, NQ], F32, kind="ExternalOutput").ap()

    spt_t3 = spt_d.rearrange("(t p) o -> t p o", p=128)  # [NIT, 128, 1024]

    with tile.TileContext(nc) as tc, ExitStack() as ctx:
        sb = ctx.enter_context(tc.tile_pool(name="sb", bufs=1))
        spt_pool = ctx.enter_context(tc.tile_pool(name="sptp", bufs=7))
        el_pool = ctx.enter_context(tc.tile_pool(name="elp", bufs=6))
        e_pool = ctx.enter_context(tc.tile_pool(name="ep", bufs=13))
        ps_t = ctx.enter_context(tc.tile_pool(name="pst", bufs=4, space="PSUM"))
        ps_m = ctx.enter_context(tc.tile_pool(name="psm", bufs=1, space="PSUM"))

        # ---- weights + Q inputs first: Q/K/V projections unblock early ----
        wqt = [sb.tile([128, D], BF16, name=f"wqt{ci}") for ci in range(2)]
        wkt = [sb.tile([128, D], BF16, name=f"wkt{ci}") for ci in range(2)]
        wvt = [sb.tile([128, D], BF16, name=f"wvt{ci}") for ci in range(2)]
        xcb = [[sb.tile([128, 512], BF16, name=f"x{ci}_{ib}") for ib in range(8)]
               for ci in range(2)]
        xq = [sb.tile([128, NQ], BF16, name=f"xq{co}") for co in range(2)]
        bq = sb.tile([128, 2], F32, name="bq")
        bk = sb.tile([128, 2], F32, name="bk")

        # x block 0 + Q-critical inputs first; everything on the sync ring in
        # criticality order (dispatch is ~0.6us/DMA serial on the sequencer).
        for ci in range(2):
            nc.sync.dma_start(xcb[ci][0][:], x_d[ci * 128:256 - 128 * (1 - ci),
                                                 0:512])
        for co in range(2):
            nc.sync.dma_start(xq[co][:], xq_d[co * 128:(co + 1) * 128, :])
        for ci in range(2):
            sl = slice(ci * 128, (ci + 1) * 128)
            nc.sync.dma_start(wqt[ci][:], wqt_d[sl, :])
            nc.sync.dma_start(wkt[ci][:], wkt_d[sl, :])
            nc.sync.dma_start(wvt[ci][:], wvt_d[sl, :])
        nc.sync.dma_start(bq[:], bq_d[:, :])
        nc.sync.dma_start(bk[:], bk_d[:, :])
        # split x-block dispatch across the sync and gpsimd rings so the
        # serial ~0.6us/dma dispatch cost halves on the critical sync queue
        for ib in range(1, 8):
            nc.sync.dma_start(xcb[0][ib][:],
                              x_d[0:128, ib * 512:(ib + 1) * 512])
            nc.gpsimd.dma_start(xcb[1][ib][:],
                                x_d[128:256, ib * 512:(ib + 1) * 512])

        k_sb = [sb.tile([128, N], BF16, name=f"k{co}") for co in range(2)]
        q_sb = [sb.tile([128, NQ], BF16, name=f"q{co}") for co in range(2)]
        vt = sb.tile([128, NIT, H, VTS], BF16, name="vt")
        nc.gpsimd.memset(vt[:, :, :, 64:65], 1.0)
        msg = [sb.tile([128, NQ], BF16, name=f"msg{co}") for co in range(2)]

        # ---- tail inputs (small; dispatched after the critical x/weights) ----
        w1t = [sb.tile([128, 128], BF16, name=f"w1t{ci}") for ci in range(2)]
        for ci in range(2):
            nc.sync.dma_start(w1t[ci][:], w1t_d[ci * 128:(ci + 1) * 128, :])
        w2t = sb.tile([128, 128], BF16, name="w2t")
        nc.sync.dma_start(w2t[:], w2t_d[:, :])
        w3t = sb.tile([128, D], BF16, name="w3t")
        nc.sync.dma_start(w3t[:], w3t_d[:, :])
        idn = sb.tile([128, 128], BF16, name="idn")
        nc.sync.dma_start(idn[:], idn_d[:, :])
        b1 = sb.tile([128, 1], F32, name="b1")
        b2 = sb.tile([128, 1], F32, name="b2")
        nc.sync.dma_start(b1[:], b1_d[:, :])
        nc.sync.dma_start(b2[:], b2_d[:, :])
        if has_bv:
            bv = sb.tile([128, 2], F32, name="bv")
            nc.sync.dma_start(bv[:], bv_d[:, :])
        if has_b3:
            b3 = sb.tile([128, 2], F32, name="b3")
            nc.sync.dma_start(b3[:], b3_d[:, :])

        # ---- PE warmup: tiny matmuls so HAM unthrottles during DMA ramp ----
        warm = sb.tile([128, 64], BF16, name="warm")
        nc.vector.memset(warm[:].bitcast(F32)[:, 0:32], 0.0)
        wps = ps_t.tile([128, NQ], F32, tag="t")
        for r in range(18):
            nc.tensor.matmul(wps[0:64, 0:64], warm[:], warm[:],
                             start=True, stop=True)

        # ---- spt prefetch: host ships [128, 1024] tiles with the key chunk
        # already replicated in both halves (flat elementwise APs, and one
        # dma_start per tile on the otherwise-idle gpsimd ring) ----
        spt_tiles = {}

        def load_spt(it):
            t = spt_pool.tile([128, 2, 512], BF16, tag="spt")
            nc.gpsimd.dma_start(t[:], spt_t3[it])
            spt_tiles[it] = t

        def load_spt_sync(it):
            t = spt_pool.tile([128, 2, 512], BF16, tag="spt")
            nc.sync.dma_start(t[:], spt_t3[it])
            spt_tiles[it] = t

        for it in range(5):
            load_spt(it)

        # ---- Q projection ----
        for co in range(2):
            ps = ps_t.tile([128, NQ], F32, tag="t")
            for ci in range(2):
                nc.tensor.matmul(ps[:],
                                 wqt[ci][:, co * 128:(co + 1) * 128],
                                 xq[ci][:],
                                 start=(ci == 0), stop=(ci == 1))
            if has_bq:
                nc.scalar.activation(q_sb[co][:], ps[:], Id,
                                     bias=bq[:, co:co + 1])
            else:
                nc.scalar.copy(q_sb[co][:], ps[:])

        # full-partition accumulator tiles: [0:65] = message+denominator,
        # [96:128] = scratch target for HAM warm-filler matmuls
        mps = [ps_m.tile([128, NQ], F32, name=f"mps{h}") for h in range(H)]

        def emit_proj_k(ib, co):
            ps = ps_t.tile([128, NQ], F32, tag="t")
            for ci in range(2):
                nc.tensor.matmul(ps[:],
                                 wkt[ci][:, co * 128:(co + 1) * 128],
                                 xcb[ci][ib][:],
                                 start=(ci == 0), stop=(ci == 1))
            ksl = k_sb[co][:, ib * 512:(ib + 1) * 512]
            if has_bk:
                nc.scalar.activation(ksl, ps[:], Id, bias=bk[:, co:co + 1])
            else:
                nc.scalar.copy(ksl, ps[:])

        def emit_proj_v(ib, w):
            # V^T: one 128-key chunk (itp = 4*ib + w)
            itp = ib * 4 + w
            vps = ps_t.tile([128, NQ], F32, tag="t")
            icol = slice((itp % 4) * 128, (itp % 4) * 128 + 128)
            for ci in range(2):
                nc.tensor.matmul(vps[:, 0:D], xcb[ci][ib][:, icol],
                                 wvt[ci][:], start=(ci == 0), stop=(ci == 1))
            vdst = vt[:, itp, :, 0:64]
            vsrc = vps[:, 0:D].rearrange("p (h c) -> p h c", h=H)
            nc.scalar.copy(vdst, vsrc)

        def emit_proj(ib):
            emit_proj_k(ib, 0)
            emit_proj_k(ib, 1)
            for w in range(4):
                emit_proj_v(ib, w)

        # message matmuls run TWO tiles behind the scores/mult/exp chain so
        # they never head-of-line block the next scores on a fresh exp
        def emit_msg(p):
            pit, ph, e2 = p
            nc.tensor.matmul(mps[ph][0:65, :], vt[:, pit, ph, 0:65],
                             e2[:], start=(pit == 0), stop=(pit == NIT - 1))

        from collections import deque
        pend = deque()

        def emit_attn(it, h):
            # one head per 1-bank psum tile: 4 pipeline slots instead of 2
            if it + 5 < NIT and h == 0:
                load_spt_sync(it + 5)
            idx = it * 4 + h
            sps = ps_t.tile([128, NQ], F32, tag="t")
            ro = (h % 2) * 64
            nc.tensor.matmul(
                sps[:],
                k_sb[h // 2][ro:ro + 64, it * 128:(it + 1) * 128],
                q_sb[h // 2][ro:ro + 64, :],
                start=True, stop=True)
            spt_t = spt_tiles[it] if h < 3 else spt_tiles.pop(it)
            el = el_pool.tile([128, NQ], BF16, tag="el")
            if idx % 8 == 1:
                # ACT drains the psum (frees the scores slot faster) and the
                # mult runs at DVE 2x off SBUF -- rebalances DVE -> ACT
                elc = el_pool.tile([128, NQ], BF16, tag="el")
                nc.scalar.copy(elc[:], sps[:])
                nc.vector.tensor_tensor(el[:], elc[:], spt_t[:, h % 2, :], Mu)
            else:
                nc.vector.tensor_tensor(el[:], sps[:], spt_t[:, h % 2, :], Mu)
            if idx % GPS_MOD == GPS_PHASE:
                ei = e_pool.tile([128, NQ], I16, tag="e")
                nc.gpsimd.tensor_scalar(ei[:], el[:], A16, B16, Mu, Ad)
                e2 = ei[:].bitcast(BF16)
            else:
                ef = e_pool.tile([128, NQ], BF16, tag="e")
                nc.scalar.activation(ef[:], el[:], Exp)
                e2 = ef[:]
            pend.append((it, h, e2))
            if len(pend) > 10:
                emit_msg(pend.popleft())

        # ---- interleaved projection + attention: proj pieces spread between
        # attn tiles so the PE never has a multi-us foreign burst ----
        emit_proj(0)
        for ib in range(1, 8):
            slots = [(it, h) for it in range(4 * (ib - 1), 4 * ib)
                     for h in range(4)]
            for i, (it, h) in enumerate(slots):
                emit_attn(it, h)
                if i == 2:
                    emit_proj_k(ib, 0)
                elif i == 4:
                    emit_proj_k(ib, 1)
                elif i == 7:
                    emit_proj_v(ib, 0)
                elif i == 9:
                    emit_proj_v(ib, 1)
                elif i == 11:
                    emit_proj_v(ib, 2)
                elif i == 13:
                    emit_proj_v(ib, 3)
        for it in range(28, 32):
            for h in range(4):
                emit_attn(it, h)

        # ---- tail, overlapped: heads 0/1 finished one tile ago, so their
        # norm chain runs while the PE emits heads 2/3's final messages and
        # then the MLP's first half ----
        dh = [sb.tile([1, NQ], F32, name=f"dh{h}") for h in range(H)]
        dbc = [sb.tile([64, NQ], F32, name=f"dbc{h}") for h in range(H)]
        rbc = [sb.tile([64, NQ], F32, name=f"rbc{h}") for h in range(H)]

        def norm_head(h):
            co, ro = h // 2, (h % 2) * 64
            nc.vector.reciprocal_approx_fast(out=rbc[h][:], in_=dbc[h][:])
            nc.vector.tensor_tensor(msg[co][ro:ro + 64, :], mps[h][0:64, :],
                                    rbc[h][:], Mu)
            if has_bv:
                nc.scalar.activation(msg[co][ro:ro + 64, :],
                                     msg[co][ro:ro + 64, :], Id,
                                     bias=bv[ro:ro + 64, co:co + 1])

        for _ in range(8):
            emit_msg(pend.popleft())
        for h in (0, 1):
            nc.scalar.copy(dh[h][:], mps[h][64:65, :])
        for h in (0, 1):
            nc.gpsimd.partition_broadcast(dbc[h][:], dh[h][:], channels=64)
        while pend:
            emit_msg(pend.popleft())
        norm_head(0)
        norm_head(1)
        for h in (2, 3):
            nc.scalar.copy(dh[h][:], mps[h][64:65, :])
        for h in (2, 3):
            nc.gpsimd.partition_broadcast(dbc[h][:], dh[h][:], channels=64)

        # ---- message MLP + residual (residual via identity matmul) ----
        u1 = ps_t.tile([128, NQ], F32, tag="t")
        nc.tensor.matmul(u1[:], w1t[0][:], msg[0][:], start=True, stop=False)
        norm_head(2)
        norm_head(3)
        nc.tensor.matmul(u1[:], w1t[1][:], msg[1][:], start=False, stop=True)
        h1 = sb.tile([128, NQ], BF16, name="h1")
        nc.scalar.activation(h1[:], u1[:], Relu, bias=b1[:, 0:1])
        u2 = ps_t.tile([128, NQ], F32, tag="t")
        nc.tensor.matmul(u2[:], w2t[:], h1[:], start=True, stop=True)
        h2 = sb.tile([128, NQ], BF16, name="h2")
        nc.scalar.activation(h2[:], u2[:], Relu, bias=b2[:, 0:1])
        for co in range(2):
            u3 = ps_t.tile([128, NQ], F32, tag="t")
            nc.tensor.matmul(u3[:], w3t[:, co * 128:(co + 1) * 128],
                             h2[:], start=True, stop=False)
            nc.tensor.matmul(u3[:], idn[:], xq[co][:],
                             start=False, stop=True)
            ot = sb.tile([128, NQ], F32, name=f"ot{co}")
            if has_b3:
                nc.scalar.activation(ot[:], u3[:], Id,
                                     bias=b3[:, co:co + 1])
            else:
                nc.scalar.copy(ot[:], u3[:])
            nc.sync.dma_start(out_d[co * 128:(co + 1) * 128, :], ot[:])

    nc.compile()
    return nc


def _prep_inputs(inputs):
    import ml_dtypes
    bf = lambda a: np.ascontiguousarray(
        np.asarray(a, dtype=np.float32).astype(ml_dtypes.bfloat16))
    f = lambda a: np.ascontiguousarray(np.asarray(a, dtype=np.float32))
    x32 = f(inputs["corr_feat_belief"][0])                  # [D, N]
    spT = np.asarray(inputs["spatial_compatibility"][0]).T  # [N(keys), N(queries)]
    Wq, bq = f(inputs["Wq"]), f(inputs["bq"])
    Wk, bk = f(inputs["Wk"]), f(inputs["bk"])
    Wv, bv = f(inputs["Wv"]), f(inputs["bv"])
    W1, b1, g1, be1 = f(inputs["W1"]), f(inputs["b1"]), f(inputs["g1"]), f(inputs["be1"])
    W2, b2, g2, be2 = f(inputs["W2"]), f(inputs["b2"]), f(inputs["g2"]), f(inputs["be2"])
    W3, b3 = f(inputs["W3"]), f(inputs["b3"])

    scale = np.float32(1.0 / np.sqrt(DH))
    s1 = (g1 / np.sqrt(np.float32(1.0) + np.float32(1e-5))).astype(np.float32)
    s2 = (g2 / np.sqrt(np.float32(1.0) + np.float32(1e-5))).astype(np.float32)

    spT_bf = bf(spT)
    x_bf = bf(x32)
    common = dict(
        x=x_bf,
        wqt=bf(Wq.T * scale),
        wkt=bf(Wk.T),
        wvt=bf(Wv.T),
        w1t=bf((W1 * s1[:, None]).T),
        w2t=bf((W2 * s2[:, None]).T),
        w3t=bf(W3.T),
        idn=bf(np.eye(128, dtype=np.float32)),
        bq2=f((bq * scale).reshape(2, 128).T),
        bk2=f(bk.reshape(2, 128).T),
        bv2=f(bv.reshape(2, 128).T),
        b1f=f((s1 * b1 + be1).reshape(128, 1)),
        b2f=f((s2 * b2 + be2).reshape(128, 1)),
        b32=f(b3.reshape(2, 128).T),
    )
    in_maps = []
    for m in range(NCORES):
        sl = slice(m * NQ, (m + 1) * NQ)
        im = dict(common)
        im["xq"] = np.ascontiguousarray(x_bf[:, sl])
        sp = np.ascontiguousarray(spT_bf[:, sl]).reshape(NIT, 128, 1, NQ)
        im["sptd"] = np.ascontiguousarray(
            np.broadcast_to(sp, (NIT, 128, 2, NQ))).reshape(N, 2 * NQ)
        in_maps.append(im)
    flags = tuple(bool(np.any(b != 0)) for b in (bq, bk, bv, b3))
    return in_maps, flags


def _run(inputs, trace=False):
    from concourse.bass_utils import run_bass_kernel_spmd
    in_maps, flags = _prep_inputs(inputs)
    if flags not in _CACHE:
        _CACHE[flags] = _build(*flags)
    nc = _CACHE[flags]
    res = run_bass_kernel_spmd(nc, in_maps, core_ids=list(range(NCORES)),
                               trace=trace)
    out = np.concatenate([res.results[m]["out"] for m in range(NCORES)],
                         axis=1)[None]
    return np.ascontiguousarray(out.astype(np.float32)), res


def kernel(**inputs):
    out, _ = _run(inputs, trace=False)
    return out


# revision 53
# speedup vs baseline: 1.0184x; 1.0184x over previous
"""Bass/Tile TRN2 kernel for a non-local attention block (BaseNonLocalBlock).

Contract: kernel(**inputs) takes the FULL inputs of the nn.Module problem
(B=1, D=256, H=4, N=4096) and returns the FULL output [1, 256, 4096].

Sharding: query columns of the N x N attention are split across the 8
NeuronCores (512 queries per core). K/V projections are computed
redundantly on every core (cheap); each core produces its own output
column slice and the host concatenates.

Per-core algorithm (flash-attention style, scores never hit HBM):
  Q = (Wq/8) @ xq                      [256, 512]  (1/sqrt(DH) folded in)
  K = Wk @ x                           [256, 4096]
  V_T = x^T @ Wv^T (+ones col/head)    [4096, 4*68] (denominator trick)
  loop over key chunks it (32 x 128 keys), head pair hp:
    S_T = K_h[:, it]^T @ Q_h           [128, 2, 512] psum (two row-packed
                                        matmuls run concurrently in the
                                        upper/lower PE sub-arrays)
    el  = S_T * spt_rep                one flat [128,1024] DVE mult (spt is
                                        DMA'd twice so no broadcast AP)
    E   = exp(el)                      ACT exp, or GPSIMD exp2 bit-trick
                                        (int16(el*184.66+16249) bitcast bf16)
                                        on a subset of tiles to offload ACT
    msg_h += V_T[it,h]^T @ E           [65, 512] psum accum; row 64 = denom
  Projections are interleaved with the attention loop (proj block ib on the
  PE while the elementwise engines process block ib-1) so the DVE -- the
  critical engine -- starts within ~4us.
  norm: recip(denominator row) on [1,512], gpsimd partition_broadcast,
  one DVE mult per head.
  out = xq + W3 @ relu(bn2(W2 @ relu(bn1(W1 @ msg))))  (BN folded into W;
  residual added on the PE via an identity matmul into the last psum).
"""

import numpy as np
from contextlib import ExitStack

D = 256
N = 4096
NQ = 512          # queries per core
H = 4
DH = 64
NCORES = 8
NIT = N // 128    # 32 key chunks
VTS = 68          # padded per-head stride in the V_T-aug tile

A16 = 184.6617    # 128*log2(e) for the exp2 bit-trick
B16 = 16256.0 - 7.0

# tiles (it, hp) with (2*it + hp) % GPS_MOD == GPS_PHASE use the GPSIMD
# bit-trick exp instead of ACT exp (load balancing)
GPS_MOD = 2
GPS_PHASE = 1

_CACHE = {}


def _build(has_bq, has_bk, has_bv, has_b3):
    import concourse.bass as bass
    import concourse.tile as tile
    from concourse import bacc, mybir

    F32 = mybir.dt.float32
    BF16 = mybir.dt.bfloat16
    I16 = mybir.dt.int16
    Id = mybir.ActivationFunctionType.Identity
    Exp = mybir.ActivationFunctionType.Exp
    Relu = mybir.ActivationFunctionType.Relu
    Mu = mybir.AluOpType.mult
    Ad = mybir.AluOpType.add

    nc = bacc.Bacc("TRN2", target_bir_lowering=False, debug=False,
                   num_devices=NCORES)

    x_d = nc.dram_tensor("x", [D, N], BF16, kind="ExternalInput").ap()
    xq_d = nc.dram_tensor("xq", [D, NQ], BF16, kind="ExternalInput").ap()
    spt_d = nc.dram_tensor("sptd", [N, 2 * NQ], BF16, kind="ExternalInput").ap()
    wqt_d = nc.dram_tensor("wqt", [D, D], BF16, kind="ExternalInput").ap()
    wkt_d = nc.dram_tensor("wkt", [D, D], BF16, kind="ExternalInput").ap()
    wvt_d = nc.dram_tensor("wvt", [D, D], BF16, kind="ExternalInput").ap()
    w1t_d = nc.dram_tensor("w1t", [D, 128], BF16, kind="ExternalInput").ap()
    w2t_d = nc.dram_tensor("w2t", [128, 128], BF16, kind="ExternalInput").ap()
    w3t_d = nc.dram_tensor("w3t", [128, D], BF16, kind="ExternalInput").ap()
    idn_d = nc.dram_tensor("idn", [128, 128], BF16, kind="ExternalInput").ap()
    bq_d = nc.dram_tensor("bq2", [128, 2], F32, kind="ExternalInput").ap()
    bk_d = nc.dram_tensor("bk2", [128, 2], F32, kind="ExternalInput").ap()
    bv_d = nc.dram_tensor("bv2", [128, 2], F32, kind="ExternalInput").ap()
    b1_d = nc.dram_tensor("b1f", [128, 1], F32, kind="ExternalInput").ap()
    b2_d = nc.dram_tensor("b2f", [128, 1], F32, kind="ExternalInput").ap()
    b3_d = nc.dram_tensor("b32", [128, 2], F32, kind="ExternalInput").ap()
    out_d = nc.dram_tensor("out", [D, NQ], F32, kind="ExternalOutput").ap()

    spt_t3 = spt_d.rearrange("(t p) o -> t p o", p=128)  # [NIT, 128, 1024]

    with tile.TileContext(nc) as tc, ExitStack() as ctx:
        sb = ctx.enter_context(tc.tile_pool(name="sb", bufs=1))
        spt_pool = ctx.enter_context(tc.tile_pool(name="sptp", bufs=7))
        el_pool = ctx.enter_context(tc.tile_pool(name="elp", bufs=6))
        e_pool = ctx.enter_context(tc.tile_pool(name="ep", bufs=11))
        ps_t = ctx.enter_context(tc.tile_pool(name="pst", bufs=4, space="PSUM"))
        ps_m = ctx.enter_context(tc.tile_pool(name="psm", bufs=1, space="PSUM"))

        # ---- weights + Q inputs first: Q/K/V projections unblock early ----
        wqt = [sb.tile([128, D], BF16, name=f"wqt{ci}") for ci in range(2)]
        wkt = [sb.tile([128, D], BF16, name=f"wkt{ci}") for ci in range(2)]
        wvt = [sb.tile([128, D], BF16, name=f"wvt{ci}") for ci in range(2)]
        xcb = [[sb.tile([128, 512], BF16, name=f"x{ci}_{ib}") for ib in range(8)]
               for ci in range(2)]
        xq = [sb.tile([128, NQ], BF16, name=f"xq{co}") for co in range(2)]
        bq = sb.tile([128, 2], F32, name="bq")
        bk = sb.tile([128, 2], F32, name="bk")

        # x block 0 + Q-critical inputs first; everything on the sync ring in
        # criticality order (dispatch is ~0.6us/DMA serial on the sequencer).
        for ci in range(2):
            nc.sync.dma_start(xcb[ci][0][:], x_d[ci * 128:256 - 128 * (1 - ci),
                                                 0:512])
        for co in range(2):
            nc.sync.dma_start(xq[co][:], xq_d[co * 128:(co + 1) * 128, :])
        for ci in range(2):
            sl = slice(ci * 128, (ci + 1) * 128)
            nc.sync.dma_start(wqt[ci][:], wqt_d[sl, :])
            nc.sync.dma_start(wkt[ci][:], wkt_d[sl, :])
            nc.sync.dma_start(wvt[ci][:], wvt_d[sl, :])
        nc.sync.dma_start(bq[:], bq_d[:, :])
        nc.sync.dma_start(bk[:], bk_d[:, :])
        # split x-block dispatch across the sync and gpsimd rings so the
        # serial ~0.6us/dma dispatch cost halves on the critical sync queue
        for ib in range(1, 8):
            nc.sync.dma_start(xcb[0][ib][:],
                              x_d[0:128, ib * 512:(ib + 1) * 512])
            nc.gpsimd.dma_start(xcb[1][ib][:],
                                x_d[128:256, ib * 512:(ib + 1) * 512])

        k_sb = [sb.tile([128, N], BF16, name=f"k{co}") for co in range(2)]
        q_sb = [sb.tile([128, NQ], BF16, name=f"q{co}") for co in range(2)]
        vt = sb.tile([128, NIT, H, VTS], BF16, name="vt")
        nc.gpsimd.memset(vt[:, :, :, 64:65], 1.0)
        msg = [sb.tile([128, NQ], BF16, name=f"msg{co}") for co in range(2)]

        # ---- tail inputs (small; dispatched after the critical x/weights) ----
        w1t = [sb.tile([128, 128], BF16, name=f"w1t{ci}") for ci in range(2)]
        for ci in range(2):
            nc.sync.dma_start(w1t[ci][:], w1t_d[ci * 128:(ci + 1) * 128, :])
        w2t = sb.tile([128, 128], BF16, name="w2t")
        nc.sync.dma_start(w2t[:], w2t_d[:, :])
        w3t = sb.tile([128, D], BF16, name="w3t")
        nc.sync.dma_start(w3t[:], w3t_d[:, :])
        idn = sb.tile([128, 128], BF16, name="idn")
        nc.sync.dma_start(idn[:], idn_d[:, :])
        b1 = sb.tile([128, 1], F32, name="b1")
        b2 = sb.tile([128, 1], F32, name="b2")
        nc.sync.dma_start(b1[:], b1_d[:, :])
        nc.sync.dma_start(b2[:], b2_d[:, :])
        if has_bv:
            bv = sb.tile([128, 2], F32, name="bv")
            nc.sync.dma_start(bv[:], bv_d[:, :])
        if has_b3:
            b3 = sb.tile([128, 2], F32, name="b3")
            nc.sync.dma_start(b3[:], b3_d[:, :])

        # ---- PE warmup: tiny matmuls so HAM unthrottles during DMA ramp ----
        warm = sb.tile([128, 64], BF16, name="warm")
        nc.vector.memset(warm[:].bitcast(F32)[:, 0:32], 0.0)
        wps = ps_t.tile([128, NQ], F32, tag="t")
        for r in range(18):
            nc.tensor.matmul(wps[0:64, 0:64], warm[:], warm[:],
                             start=True, stop=True)

        # ---- spt prefetch: host ships [128, 1024] tiles with the key chunk
        # already replicated in both halves (flat elementwise APs, and one
        # dma_start per tile on the otherwise-idle gpsimd ring) ----
        spt_tiles = {}

        def load_spt(it):
            t = spt_pool.tile([128, 2, 512], BF16, tag="spt")
            nc.gpsimd.dma_start(t[:], spt_t3[it])
            spt_tiles[it] = t

        def load_spt_sync(it):
            t = spt_pool.tile([128, 2, 512], BF16, tag="spt")
            nc.sync.dma_start(t[:], spt_t3[it])
            spt_tiles[it] = t

        for it in range(5):
            load_spt(it)

        # ---- Q projection ----
        for co in range(2):
            ps = ps_t.tile([128, NQ], F32, tag="t")
            for ci in range(2):
                nc.tensor.matmul(ps[:],
                                 wqt[ci][:, co * 128:(co + 1) * 128],
                                 xq[ci][:],
                                 start=(ci == 0), stop=(ci == 1))
            if has_bq:
                nc.scalar.activation(q_sb[co][:], ps[:], Id,
                                     bias=bq[:, co:co + 1])
            else:
                nc.scalar.copy(q_sb[co][:], ps[:])

        # full-partition accumulator tiles: [0:65] = message+denominator,
        # [96:128] = scratch target for HAM warm-filler matmuls
        mps = [ps_m.tile([128, NQ], F32, name=f"mps{h}") for h in range(H)]

        def emit_proj_k(ib, co):
            ps = ps_t.tile([128, NQ], F32, tag="t")
            for ci in range(2):
                nc.tensor.matmul(ps[:],
                                 wkt[ci][:, co * 128:(co + 1) * 128],
                                 xcb[ci][ib][:],
                                 start=(ci == 0), stop=(ci == 1))
            ksl = k_sb[co][:, ib * 512:(ib + 1) * 512]
            if has_bk:
                nc.scalar.activation(ksl, ps[:], Id, bias=bk[:, co:co + 1])
            else:
                nc.scalar.copy(ksl, ps[:])

        def emit_proj_v(ib, w):
            # V^T: one 128-key chunk (itp = 4*ib + w)
            itp = ib * 4 + w
            vps = ps_t.tile([128, NQ], F32, tag="t")
            icol = slice((itp % 4) * 128, (itp % 4) * 128 + 128)
            for ci in range(2):
                nc.tensor.matmul(vps[:, 0:D], xcb[ci][ib][:, icol],
                                 wvt[ci][:], start=(ci == 0), stop=(ci == 1))
            vdst = vt[:, itp, :, 0:64]
            vsrc = vps[:, 0:D].rearrange("p (h c) -> p h c", h=H)
            nc.scalar.copy(vdst, vsrc)

        def emit_proj(ib):
            emit_proj_k(ib, 0)
            emit_proj_k(ib, 1)
            for w in range(4):
                emit_proj_v(ib, w)

        # message matmuls run TWO tiles behind the scores/mult/exp chain so
        # they never head-of-line block the next scores on a fresh exp
        def emit_msg(p):
            pit, ph, e2 = p
            nc.tensor.matmul(mps[ph][0:65, :], vt[:, pit, ph, 0:65],
                             e2[:], start=(pit == 0), stop=(pit == NIT - 1))

        from collections import deque
        pend = deque()

        def emit_attn(it, h):
            # one head per 1-bank psum tile: 4 pipeline slots instead of 2
            if it + 5 < NIT and h == 0:
                load_spt_sync(it + 5)
            idx = it * 4 + h
            sps = ps_t.tile([128, NQ], F32, tag="t")
            ro = (h % 2) * 64
            nc.tensor.matmul(
                sps[:],
                k_sb[h // 2][ro:ro + 64, it * 128:(it + 1) * 128],
                q_sb[h // 2][ro:ro + 64, :],
                start=True, stop=True)
            spt_t = spt_tiles[it] if h < 3 else spt_tiles.pop(it)
            el = el_pool.tile([128, NQ], BF16, tag="el")
            if idx % 8 == 1:
                # ACT drains the psum (frees the scores slot faster) and the
                # mult runs at DVE 2x off SBUF -- rebalances DVE -> ACT
                elc = el_pool.tile([128, NQ], BF16, tag="el")
                nc.scalar.copy(elc[:], sps[:])
                nc.vector.tensor_tensor(el[:], elc[:], spt_t[:, h % 2, :], Mu)
            else:
                nc.vector.tensor_tensor(el[:], sps[:], spt_t[:, h % 2, :], Mu)
            if idx % GPS_MOD == GPS_PHASE:
                ei = e_pool.tile([128, NQ], I16, tag="e")
                nc.gpsimd.tensor_scalar(ei[:], el[:], A16, B16, Mu, Ad)
                e2 = ei[:].bitcast(BF16)
            else:
                ef = e_pool.tile([128, NQ], BF16, tag="e")
                nc.scalar.activation(ef[:], el[:], Exp)
                e2 = ef[:]
            pend.append((it, h, e2))
            if len(pend) > 8:
                emit_msg(pend.popleft())

        # ---- interleaved projection + attention: proj pieces spread between
        # attn tiles so the PE never has a multi-us foreign burst ----
        emit_proj(0)
        for ib in range(1, 8):
            slots = [(it, h) for it in range(4 * (ib - 1), 4 * ib)
                     for h in range(4)]
            for i, (it, h) in enumerate(slots):
                emit_attn(it, h)
                if i == 2:
                    emit_proj_k(ib, 0)
                elif i == 4:
                    emit_proj_k(ib, 1)
                elif i == 7:
                    emit_proj_v(ib, 0)
                elif i == 9:
                    emit_proj_v(ib, 1)
                elif i == 11:
                    emit_proj_v(ib, 2)
                elif i == 13:
                    emit_proj_v(ib, 3)
        for it in range(28, 32):
            for h in range(4):
                emit_attn(it, h)

        # ---- tail, overlapped: heads 0/1 finished one tile ago, so their
        # norm chain runs while the PE emits heads 2/3's final messages and
        # then the MLP's first half ----
        dh = [sb.tile([1, NQ], F32, name=f"dh{h}") for h in range(H)]
        dbc = [sb.tile([64, NQ], F32, name=f"dbc{h}") for h in range(H)]
        rbc = [sb.tile([64, NQ], F32, name=f"rbc{h}") for h in range(H)]

        def norm_head(h):
            co, ro = h // 2, (h % 2) * 64
            nc.vector.reciprocal_approx_fast(out=rbc[h][:], in_=dbc[h][:])
            nc.vector.tensor_tensor(msg[co][ro:ro + 64, :], mps[h][0:64, :],
                                    rbc[h][:], Mu)
            if has_bv:
                nc.scalar.activation(msg[co][ro:ro + 64, :],
                                     msg[co][ro:ro + 64, :], Id,
                                     bias=bv[ro:ro + 64, co:co + 1])

        for _ in range(6):
            emit_msg(pend.popleft())
        for h in (0, 1):
            nc.scalar.copy(dh[h][:], mps[h][64:65, :])
        for h in (0, 1):
            nc.gpsimd.partition_broadcast(dbc[h][:], dh[h][:], channels=64)
        while pend:
            emit_msg(pend.popleft())
        norm_head(0)
        norm_head(1)
        for h in (2, 3):
            nc.scalar.copy(dh[h][:], mps[h][64:65, :])
        for h in (2, 3):
            nc.gpsimd.partition_broadcast(dbc[h][:], dh[h][:], channels=64)

        # ---- message MLP + residual (residual via identity matmul) ----
        u1 = ps_t.tile([128, NQ], F32, tag="t")
        nc.tensor.matmul(u1[:], w1t[0][:], msg[0][:], start=True, stop=False)
        norm_head(2)
        norm_head(3)
        nc.tensor.matmul(u1[:], w1t[1][:], msg[1][:], start=False, stop=True)
        h1 = sb.tile([128, NQ], BF16, name="h1")
        nc.scalar.activation(h1[:], u1[:], Relu, bias=b1[:, 0:1])
        u2 = ps_t.tile([128, NQ], F32, tag="t")
        nc.tensor.matmul(u2[:], w2t[:], h1[:], start=True, stop=True)
        h2 = sb.tile([128, NQ], BF16, name="h2")
        nc.scalar.activation(h2[:], u2[:], Relu, bias=b2[:, 0:1])
        for co in range(2):
            u3 = ps_t.tile([128, NQ], F32, tag="t")
            nc.tensor.matmul(u3[:], w3t[:, co * 128:(co + 1) * 128],
                             h2[:], start=True, stop=False)
            nc.tensor.matmul(u3[:], idn[:], xq[co][:],
                             start=False, stop=True)
            ot = sb.tile([128, NQ], F32, name=f"ot{co}")
            if has_b3:
                nc.scalar.activation(ot[:], u3[:], Id,
                                     bias=b3[:, co:co + 1])
            else:
                nc.scalar.copy(ot[:], u3[:])
            nc.sync.dma_start(out_d[co * 128:(co + 1) * 128, :], ot[:])

    nc.compile()
    return nc


def _prep_inputs(inputs):
    import ml_dtypes
    bf = lambda a: np.ascontiguousarray(
        np.asarray(a, dtype=np.float32).astype(ml_dtypes.bfloat16))
    f = lambda a: np.ascontiguousarray(np.asarray(a, dtype=np.float32))
    x32 = f(inputs["corr_feat_belief"][0])                  # [D, N]
    spT = np.asarray(inputs["spatial_compatibility"][0]).T  # [N(keys), N(queries)]
    Wq, bq = f(inputs["Wq"]), f(inputs["bq"])
    Wk, bk = f(inputs["Wk"]), f(inputs["bk"])
    Wv, bv = f(inputs["Wv"]), f(inputs["bv"])
    W1, b1, g1, be1 = f(inputs["W1"]), f(inputs["b1"]), f(inputs["g1"]), f(inputs["be1"])
    W2, b2, g2, be2 = f(inputs["W2"]), f(inputs["b2"]), f(inputs["g2"]), f(inputs["be2"])
    W3, b3 = f(inputs["W3"]), f(inputs["b3"])

    scale = np.float32(1.0 / np.sqrt(DH))
    s1 = (g1 / np.sqrt(np.float32(1.0) + np.float32(1e-5))).astype(np.float32)
    s2 = (g2 / np.sqrt(np.float32(1.0) + np.float32(1e-5))).astype(np.float32)

    spT_bf = bf(spT)
    x_bf = bf(x32)
    common = dict(
        x=x_bf,
        wqt=bf(Wq.T * scale),
        wkt=bf(Wk.T),
        wvt=bf(Wv.T),
        w1t=bf((W1 * s1[:, None]).T),
        w2t=bf((W2 * s2[:, None]).T),
        w3t=bf(W3.T),
        idn=bf(np.eye(128, dtype=np.float32)),
        bq2=f((bq * scale).reshape(2, 128).T),
        bk2=f(bk.reshape(2, 128).T),
        bv2=f(bv.reshape(2, 128).T),
        b1f=f((s1 * b1 + be1).reshape(128, 1)),
        b2f=f((s2 * b2 + be2).reshape(128, 1)),
        b32=f(b3.reshape(2, 128).T),
    )
    in_maps = []
    for m in range(NCORES):
        sl = slice(m * NQ, (m + 1) * NQ)
        im = dict(common)
        im["xq"] = np.ascontiguousarray(x_bf[:, sl])
        sp = np.ascontiguousarray(spT_bf[:, sl]).reshape(NIT, 128, 1, NQ)
        im["sptd"] = np.ascontiguousarray(
            np.broadcast_to(sp, (NIT, 128, 2, NQ))).reshape(N, 2 * NQ)
        in_maps.append(im)
    flags = tuple(bool(np.any(b != 0)) for b in (bq, bk, bv, b3))
    return in_maps, flags


def _run(inputs, trace=False):
    from concourse.bass_utils import run_bass_kernel_spmd
    in_maps, flags = _prep_inputs(inputs)
    if flags not in _CACHE:
        _CACHE[flags] = _build(*flags)
    nc = _CACHE[flags]
    res = run_bass_kernel_spmd(nc, in_maps, core_ids=list(range(NCORES)),
                               trace=trace)
    out = np.concatenate([res.results[m]["out"] for m in range(NCORES)],
                         axis=1)[None]
    return np.ascontiguousarray(out.astype(np.float32)), res


def kernel(**inputs):
    out, _ = _run(inputs, trace=False)
    return out


# revision 54
# speedup vs baseline: 1.0338x; 1.0151x over previous
"""Bass/Tile TRN2 kernel for a non-local attention block (BaseNonLocalBlock).

Contract: kernel(**inputs) takes the FULL inputs of the nn.Module problem
(B=1, D=256, H=4, N=4096) and returns the FULL output [1, 256, 4096].

Sharding: query columns of the N x N attention are split across the 8
NeuronCores (512 queries per core). K/V projections are computed
redundantly on every core (cheap); each core produces its own output
column slice and the host concatenates.

Per-core algorithm (flash-attention style, scores never hit HBM):
  Q = (Wq/8) @ xq                      [256, 512]  (1/sqrt(DH) folded in)
  K = Wk @ x                           [256, 4096]
  V_T = x^T @ Wv^T (+ones col/head)    [4096, 4*68] (denominator trick)
  loop over key chunks it (32 x 128 keys), head pair hp:
    S_T = K_h[:, it]^T @ Q_h           [128, 2, 512] psum (two row-packed
                                        matmuls run concurrently in the
                                        upper/lower PE sub-arrays)
    el  = S_T * spt_rep                one flat [128,1024] DVE mult (spt is
                                        DMA'd twice so no broadcast AP)
    E   = exp(el)                      ACT exp, or GPSIMD exp2 bit-trick
                                        (int16(el*184.66+16249) bitcast bf16)
                                        on a subset of tiles to offload ACT
    msg_h += V_T[it,h]^T @ E           [65, 512] psum accum; row 64 = denom
  Projections are interleaved with the attention loop (proj block ib on the
  PE while the elementwise engines process block ib-1) so the DVE -- the
  critical engine -- starts within ~4us.
  norm: recip(denominator row) on [1,512], gpsimd partition_broadcast,
  one DVE mult per head.
  out = xq + W3 @ relu(bn2(W2 @ relu(bn1(W1 @ msg))))  (BN folded into W;
  residual added on the PE via an identity matmul into the last psum).
"""

import numpy as np
from contextlib import ExitStack

D = 256
N = 4096
NQ = 512          # queries per core
H = 4
DH = 64
NCORES = 8
NIT = N // 128    # 32 key chunks
VTS = 68          # padded per-head stride in the V_T-aug tile

A16 = 184.6617    # 128*log2(e) for the exp2 bit-trick
B16 = 16256.0 - 7.0

# tiles (it, hp) with (2*it + hp) % GPS_MOD == GPS_PHASE use the GPSIMD
# bit-trick exp instead of ACT exp (load balancing)
GPS_MOD = 2
GPS_PHASE = 1

_CACHE = {}


def _build(has_bq, has_bk, has_bv, has_b3):
    import concourse.bass as bass
    import concourse.tile as tile
    from concourse import bacc, mybir

    F32 = mybir.dt.float32
    BF16 = mybir.dt.bfloat16
    I16 = mybir.dt.int16
    Id = mybir.ActivationFunctionType.Identity
    Exp = mybir.ActivationFunctionType.Exp
    Relu = mybir.ActivationFunctionType.Relu
    Mu = mybir.AluOpType.mult
    Ad = mybir.AluOpType.add

    nc = bacc.Bacc("TRN2", target_bir_lowering=False, debug=False,
                   num_devices=NCORES)

    x_d = nc.dram_tensor("x", [D, N], BF16, kind="ExternalInput").ap()
    xq_d = nc.dram_tensor("xq", [D, NQ], BF16, kind="ExternalInput").ap()
    spt_d = nc.dram_tensor("sptd", [N, 2 * NQ], BF16, kind="ExternalInput").ap()
    wqt_d = nc.dram_tensor("wqt", [D, D], BF16, kind="ExternalInput").ap()
    wkt_d = nc.dram_tensor("wkt", [D, D], BF16, kind="ExternalInput").ap()
    wvt_d = nc.dram_tensor("wvt", [D, D], BF16, kind="ExternalInput").ap()
    w1t_d = nc.dram_tensor("w1t", [D, 128], BF16, kind="ExternalInput").ap()
    w2t_d = nc.dram_tensor("w2t", [128, 128], BF16, kind="ExternalInput").ap()
    w3t_d = nc.dram_tensor("w3t", [128, D], BF16, kind="ExternalInput").ap()
    idn_d = nc.dram_tensor("idn", [128, 128], BF16, kind="ExternalInput").ap()
    bq_d = nc.dram_tensor("bq2", [128, 2], F32, kind="ExternalInput").ap()
    bk_d = nc.dram_tensor("bk2", [128, 2], F32, kind="ExternalInput").ap()
    bv_d = nc.dram_tensor("bv2", [128, 2], F32, kind="ExternalInput").ap()
    b1_d = nc.dram_tensor("b1f", [128, 1], F32, kind="ExternalInput").ap()
    b2_d = nc.dram_tensor("b2f", [128, 1], F32, kind="ExternalInput").ap()
    b3_d = nc.dram_tensor("b32", [128, 2], F32, kind="ExternalInput").ap()
    out_d = nc.dram_tensor("out", [D, NQ], F32, kind="ExternalOutput").ap()

    spt_t3 = spt_d.rearrange("(t p) o -> t p o", p=128)  # [NIT, 128, 1024]

    with tile.TileContext(nc) as tc, ExitStack() as ctx:
        sb = ctx.enter_context(tc.tile_pool(name="sb", bufs=1))
        spt_pool = ctx.enter_context(tc.tile_pool(name="sptp", bufs=7))
        el_pool = ctx.enter_context(tc.tile_pool(name="elp", bufs=6))
        e_pool = ctx.enter_context(tc.tile_pool(name="ep", bufs=11))
        ps_t = ctx.enter_context(tc.tile_pool(name="pst", bufs=4, space="PSUM"))
        ps_m = ctx.enter_context(tc.tile_pool(name="psm", bufs=1, space="PSUM"))

        # ---- weights + Q inputs first: Q/K/V projections unblock early ----
        wqt = [sb.tile([128, D], BF16, name=f"wqt{ci}") for ci in range(2)]
        wkt = [sb.tile([128, D], BF16, name=f"wkt{ci}") for ci in range(2)]
        wvt = [sb.tile([128, D], BF16, name=f"wvt{ci}") for ci in range(2)]
        xcb = [[sb.tile([128, 512], BF16, name=f"x{ci}_{ib}") for ib in range(8)]
               for ci in range(2)]
        xq = [sb.tile([128, NQ], BF16, name=f"xq{co}") for co in range(2)]
        bq = sb.tile([128, 2], F32, name="bq")
        bk = sb.tile([128, 2], F32, name="bk")

        # x block 0 + Q-critical inputs first; everything on the sync ring in
        # criticality order (dispatch is ~0.6us/DMA serial on the sequencer).
        for ci in range(2):
            nc.sync.dma_start(xcb[ci][0][:], x_d[ci * 128:256 - 128 * (1 - ci),
                                                 0:512])
        for co in range(2):
            nc.sync.dma_start(xq[co][:], xq_d[co * 128:(co + 1) * 128, :])
        for ci in range(2):
            sl = slice(ci * 128, (ci + 1) * 128)
            nc.sync.dma_start(wqt[ci][:], wqt_d[sl, :])
            nc.sync.dma_start(wkt[ci][:], wkt_d[sl, :])
            nc.sync.dma_start(wvt[ci][:], wvt_d[sl, :])
        nc.sync.dma_start(bq[:], bq_d[:, :])
        nc.sync.dma_start(bk[:], bk_d[:, :])
        # split x-block dispatch across the sync and gpsimd rings so the
        # serial ~0.6us/dma dispatch cost halves on the critical sync queue
        for ib in range(1, 8):
            nc.sync.dma_start(xcb[0][ib][:],
                              x_d[0:128, ib * 512:(ib + 1) * 512])
            nc.gpsimd.dma_start(xcb[1][ib][:],
                                x_d[128:256, ib * 512:(ib + 1) * 512])

        k_sb = [sb.tile([128, N], BF16, name=f"k{co}") for co in range(2)]
        q_sb = [sb.tile([128, NQ], BF16, name=f"q{co}") for co in range(2)]
        vt = sb.tile([128, NIT, H, VTS], BF16, name="vt")
        nc.gpsimd.memset(vt[:, :, :, 64:65], 1.0)
        msg = [sb.tile([128, NQ], BF16, name=f"msg{co}") for co in range(2)]

        # ---- tail inputs (small; dispatched after the critical x/weights) ----
        w1t = [sb.tile([128, 128], BF16, name=f"w1t{ci}") for ci in range(2)]
        for ci in range(2):
            nc.sync.dma_start(w1t[ci][:], w1t_d[ci * 128:(ci + 1) * 128, :])
        w2t = sb.tile([128, 128], BF16, name="w2t")
        nc.sync.dma_start(w2t[:], w2t_d[:, :])
        w3t = sb.tile([128, D], BF16, name="w3t")
        nc.sync.dma_start(w3t[:], w3t_d[:, :])
        idn = sb.tile([128, 128], BF16, name="idn")
        nc.sync.dma_start(idn[:], idn_d[:, :])
        b1 = sb.tile([128, 1], F32, name="b1")
        b2 = sb.tile([128, 1], F32, name="b2")
        nc.sync.dma_start(b1[:], b1_d[:, :])
        nc.sync.dma_start(b2[:], b2_d[:, :])
        if has_bv:
            bv = sb.tile([128, 2], F32, name="bv")
            nc.sync.dma_start(bv[:], bv_d[:, :])
        if has_b3:
            b3 = sb.tile([128, 2], F32, name="b3")
            nc.sync.dma_start(b3[:], b3_d[:, :])

        # ---- PE warmup: tiny matmuls so HAM unthrottles during DMA ramp ----
        warm = sb.tile([128, 64], BF16, name="warm")
        nc.vector.memset(warm[:].bitcast(F32)[:, 0:32], 0.0)
        wps = ps_t.tile([128, NQ], F32, tag="t")
        for r in range(18):
            nc.tensor.matmul(wps[0:64, 0:64], warm[:], warm[:],
                             start=True, stop=True)

        # ---- spt prefetch: host ships [128, 1024] tiles with the key chunk
        # already replicated in both halves (flat elementwise APs, and one
        # dma_start per tile on the otherwise-idle gpsimd ring) ----
        spt_tiles = {}

        def load_spt(it):
            t = spt_pool.tile([128, 2, 512], BF16, tag="spt")
            nc.gpsimd.dma_start(t[:], spt_t3[it])
            spt_tiles[it] = t

        def load_spt_sync(it):
            t = spt_pool.tile([128, 2, 512], BF16, tag="spt")
            nc.sync.dma_start(t[:], spt_t3[it])
            spt_tiles[it] = t

        for it in range(5):
            load_spt(it)

        # ---- Q projection ----
        for co in range(2):
            ps = ps_t.tile([128, NQ], F32, tag="t")
            for ci in range(2):
                nc.tensor.matmul(ps[:],
                                 wqt[ci][:, co * 128:(co + 1) * 128],
                                 xq[ci][:],
                                 start=(ci == 0), stop=(ci == 1))
            if has_bq:
                nc.scalar.activation(q_sb[co][:], ps[:], Id,
                                     bias=bq[:, co:co + 1])
            else:
                nc.scalar.copy(q_sb[co][:], ps[:])

        # full-partition accumulator tiles: [0:65] = message+denominator,
        # [96:128] = scratch target for HAM warm-filler matmuls
        mps = [ps_m.tile([128, NQ], F32, name=f"mps{h}") for h in range(H)]

        def emit_proj_k(ib, co):
            ps = ps_t.tile([128, NQ], F32, tag="t")
            for ci in range(2):
                nc.tensor.matmul(ps[:],
                                 wkt[ci][:, co * 128:(co + 1) * 128],
                                 xcb[ci][ib][:],
                                 start=(ci == 0), stop=(ci == 1))
            ksl = k_sb[co][:, ib * 512:(ib + 1) * 512]
            if has_bk:
                nc.scalar.activation(ksl, ps[:], Id, bias=bk[:, co:co + 1])
            else:
                nc.scalar.copy(ksl, ps[:])

        def emit_proj_v(ib, w):
            # V^T: one 128-key chunk (itp = 4*ib + w)
            itp = ib * 4 + w
            vps = ps_t.tile([128, NQ], F32, tag="t")
            icol = slice((itp % 4) * 128, (itp % 4) * 128 + 128)
            for ci in range(2):
                nc.tensor.matmul(vps[:, 0:D], xcb[ci][ib][:, icol],
                                 wvt[ci][:], start=(ci == 0), stop=(ci == 1))
            vdst = vt[:, itp, :, 0:64]
            vsrc = vps[:, 0:D].rearrange("p (h c) -> p h c", h=H)
            nc.scalar.copy(vdst, vsrc)

        def emit_proj(ib):
            emit_proj_k(ib, 0)
            emit_proj_k(ib, 1)
            for w in range(4):
                emit_proj_v(ib, w)

        # message matmuls run TWO tiles behind the scores/mult/exp chain so
        # they never head-of-line block the next scores on a fresh exp
        def emit_msg(p):
            pit, ph, e2 = p
            nc.tensor.matmul(mps[ph][0:65, :], vt[:, pit, ph, 0:65],
                             e2[:], start=(pit == 0), stop=(pit == NIT - 1))

        from collections import deque
        pend = deque()

        def emit_attn(it, h):
            # one head per 1-bank psum tile: 4 pipeline slots instead of 2
            if it + 5 < NIT and h == 0:
                load_spt_sync(it + 5)
            idx = it * 4 + h
            sps = ps_t.tile([128, NQ], F32, tag="t")
            ro = (h % 2) * 64
            nc.tensor.matmul(
                sps[:],
                k_sb[h // 2][ro:ro + 64, it * 128:(it + 1) * 128],
                q_sb[h // 2][ro:ro + 64, :],
                start=True, stop=True)
            spt_t = spt_tiles[it] if h < 3 else spt_tiles.pop(it)
            el = el_pool.tile([128, NQ], BF16, tag="el")
            if idx % 8 == 1 and it < 30:
                # ACT drains the psum (frees the scores slot faster) and the
                # mult runs at DVE 2x off SBUF -- rebalances DVE -> ACT
                elc = el_pool.tile([128, NQ], BF16, tag="el")
                nc.scalar.copy(elc[:], sps[:])
                nc.vector.tensor_tensor(el[:], elc[:], spt_t[:, h % 2, :], Mu)
            else:
                nc.vector.tensor_tensor(el[:], sps[:], spt_t[:, h % 2, :], Mu)
            if it >= 31:
                # wind-down: exp on the (by now idle) DVE so the final
                # messages never queue behind ACT/GPS at the tail
                ei = e_pool.tile([128, NQ], I16, tag="e")
                nc.vector.tensor_scalar(ei[:], el[:], A16, B16, Mu, Ad)
                e2 = ei[:].bitcast(BF16)
            elif idx % GPS_MOD == GPS_PHASE:
                ei = e_pool.tile([128, NQ], I16, tag="e")
                nc.gpsimd.tensor_scalar(ei[:], el[:], A16, B16, Mu, Ad)
                e2 = ei[:].bitcast(BF16)
            else:
                ef = e_pool.tile([128, NQ], BF16, tag="e")
                nc.scalar.activation(ef[:], el[:], Exp)
                e2 = ef[:]
            pend.append((it, h, e2))
            if len(pend) > 8:
                emit_msg(pend.popleft())

        # ---- interleaved projection + attention: proj pieces spread between
        # attn tiles so the PE never has a multi-us foreign burst ----
        emit_proj(0)
        for ib in range(1, 8):
            slots = [(it, h) for it in range(4 * (ib - 1), 4 * ib)
                     for h in range(4)]
            for i, (it, h) in enumerate(slots):
                emit_attn(it, h)
                if i == 2:
                    emit_proj_k(ib, 0)
                elif i == 4:
                    emit_proj_k(ib, 1)
                elif i == 7:
                    emit_proj_v(ib, 0)
                elif i == 9:
                    emit_proj_v(ib, 1)
                elif i == 11:
                    emit_proj_v(ib, 2)
                elif i == 13:
                    emit_proj_v(ib, 3)
        for it in range(28, 32):
            for h in range(4):
                emit_attn(it, h)

        # ---- tail, overlapped: heads 0/1 finished one tile ago, so their
        # norm chain runs while the PE emits heads 2/3's final messages and
        # then the MLP's first half ----
        dh = [sb.tile([1, NQ], F32, name=f"dh{h}") for h in range(H)]
        dbc = [sb.tile([64, NQ], F32, name=f"dbc{h}") for h in range(H)]
        rbc = [sb.tile([64, NQ], F32, name=f"rbc{h}") for h in range(H)]

        def norm_head(h):
            co, ro = h // 2, (h % 2) * 64
            nc.vector.reciprocal_approx_fast(out=rbc[h][:], in_=dbc[h][:])
            nc.vector.tensor_tensor(msg[co][ro:ro + 64, :], mps[h][0:64, :],
                                    rbc[h][:], Mu)
            if has_bv:
                nc.scalar.activation(msg[co][ro:ro + 64, :],
                                     msg[co][ro:ro + 64, :], Id,
                                     bias=bv[ro:ro + 64, co:co + 1])

        for _ in range(6):
            emit_msg(pend.popleft())
        for h in (0, 1):
            nc.scalar.copy(dh[h][:], mps[h][64:65, :])
        for h in (0, 1):
            nc.gpsimd.partition_broadcast(dbc[h][:], dh[h][:], channels=64)
        while pend:
            emit_msg(pend.popleft())
        norm_head(0)
        norm_head(1)
        for h in (2, 3):
            nc.scalar.copy(dh[h][:], mps[h][64:65, :])
        for h in (2, 3):
            nc.gpsimd.partition_broadcast(dbc[h][:], dh[h][:], channels=64)

        # ---- message MLP + residual (residual via identity matmul) ----
        u1 = ps_t.tile([128, NQ], F32, tag="t")
        nc.tensor.matmul(u1[:], w1t[0][:], msg[0][:], start=True, stop=False)
        norm_head(2)
        norm_head(3)
        nc.tensor.matmul(u1[:], w1t[1][:], msg[1][:], start=False, stop=True)
        h1 = sb.tile([128, NQ], BF16, name="h1")
        nc.scalar.activation(h1[:], u1[:], Relu, bias=b1[:, 0:1])
        u2 = ps_t.tile([128, NQ], F32, tag="t")
        nc.tensor.matmul(u2[:], w2t[:], h1[:], start=True, stop=True)
        h2 = sb.tile([128, NQ], BF16, name="h2")
        nc.scalar.activation(h2[:], u2[:], Relu, bias=b2[:, 0:1])
        for co in range(2):
            u3 = ps_t.tile([128, NQ], F32, tag="t")
            nc.tensor.matmul(u3[:], w3t[:, co * 128:(co + 1) * 128],
                             h2[:], start=True, stop=False)
            nc.tensor.matmul(u3[:], idn[:], xq[co][:],
                             start=False, stop=True)
            ot = sb.tile([128, NQ], F32, name=f"ot{co}")
            if has_b3:
                nc.scalar.activation(ot[:], u3[:], Id,
                                     bias=b3[:, co:co + 1])
            else:
                nc.scalar.copy(ot[:], u3[:])
            nc.sync.dma_start(out_d[co * 128:(co + 1) * 128, :], ot[:])

    nc.compile()
    return nc


def _prep_inputs(inputs):
    import ml_dtypes
    bf = lambda a: np.ascontiguousarray(
        np.asarray(a, dtype=np.float32).astype(ml_dtypes.bfloat16))
    f = lambda a: np.ascontiguousarray(np.asarray(a, dtype=np.float32))
    x32 = f(inputs["corr_feat_belief"][0])                  # [D, N]
    spT = np.asarray(inputs["spatial_compatibility"][0]).T  # [N(keys), N(queries)]
    Wq, bq = f(inputs["Wq"]), f(inputs["bq"])
    Wk, bk = f(inputs["Wk"]), f(inputs["bk"])
    Wv, bv = f(inputs["Wv"]), f(inputs["bv"])
    W1, b1, g1, be1 = f(inputs["W1"]), f(inputs["b1"]), f(inputs["g1"]), f(inputs["be1"])
    W2, b2, g2, be2 = f(inputs["W2"]), f(inputs["b2"]), f(inputs["g2"]), f(inputs["be2"])
    W3, b3 = f(inputs["W3"]), f(inputs["b3"])

    scale = np.float32(1.0 / np.sqrt(DH))
    s1 = (g1 / np.sqrt(np.float32(1.0) + np.float32(1e-5))).astype(np.float32)
    s2 = (g2 / np.sqrt(np.float32(1.0) + np.float32(1e-5))).astype(np.float32)

    spT_bf = bf(spT)
    x_bf = bf(x32)
    common = dict(
        x=x_bf,
        wqt=bf(Wq.T * scale),
        wkt=bf(Wk.T),
        wvt=bf(Wv.T),
        w1t=bf((W1 * s1[:, None]).T),
        w2t=bf((W2 * s2[:, None]).T),
        w3t=bf(W3.T),
        idn=bf(np.eye(128, dtype=np.float32)),
        bq2=f((bq * scale).reshape(2, 128).T),
        bk2=f(bk.reshape(2, 128).T),
        bv2=f(bv.reshape(2, 128).T),
        b1f=f((s1 * b1 + be1).reshape(128, 1)),
        b2f=f((s2 * b2 + be2).reshape(128, 1)),
        b32=f(b3.reshape(2, 128).T),
    )
    in_maps = []
    for m in range(NCORES):
        sl = slice(m * NQ, (m + 1) * NQ)
        im = dict(common)
        im["xq"] = np.ascontiguousarray(x_bf[:, sl])
        sp = np.ascontiguousarray(spT_bf[:, sl]).reshape(NIT, 128, 1, NQ)
        im["sptd"] = np.ascontiguousarray(
            np.broadcast_to(sp, (NIT, 128, 2, NQ))).reshape(N, 2 * NQ)
        in_maps.append(im)
    flags = tuple(bool(np.any(b != 0)) for b in (bq, bk, bv, b3))
    return in_maps, flags


def _run(inputs, trace=False):
    from concourse.bass_utils import run_bass_kernel_spmd
    in_maps, flags = _prep_inputs(inputs)
    if flags not in _CACHE:
        _CACHE[flags] = _build(*flags)
    nc = _CACHE[flags]
    res = run_bass_kernel_spmd(nc, in_maps, core_ids=list(range(NCORES)),
                               trace=trace)
    out = np.concatenate([res.results[m]["out"] for m in range(NCORES)],
                         axis=1)[None]
    return np.ascontiguousarray(out.astype(np.float32)), res


def kernel(**inputs):
    out, _ = _run(inputs, trace=False)
    return out
